# revision 2
# baseline (speedup 1.0000x reference)
"""GQA self-attention block (q/k/v proj + causal softmax attention + o proj)
on 8 trn2 NeuronCores — bf16 datapath, phase-interleaved.

Sharding: batch (2) x query-head-groups (4) -> 8 cores. Core c handles
batch b=c//4 and heads [8g, 8g+8) where g=c%4 (kv heads [2g, 2g+2)).
Each core computes a partial output [T, D] = ctx_heads @ o_proj_cols.T;
the host sums the 4 partials per batch.

Design (driven by hw microbenchmarks):
  - PE streams ~1 cycle/row at 2.4 GHz for bf16 regardless of stationary
    reloads, but only when its exec queue never drains; all operands bf16.
  - k stationaries are zero-padded to 128 rows (kpz[kv][:,h2,:] has the
    64 k-dims at partitions 64*h2.. and zeros elsewhere) so S matmuls use
    full-array mode — no tile_position switching against the AV matmuls.
  - causal mask = zeroing E after exp with bf16 tri01/DVE muls.
  - exp: one ACT instruction per key chunk ([128, 2, 512-128*o] AP).
  - emission interleaves: token-block tb feeds attention column a=tb as
    soon as it lands, so ACT exp overlaps phase-1 PE work and phase-3
    (o-proj) fills PE slack during ACT-bound attention stretches.
"""

import os
import sys

sys.path.insert(0, "/opt/trn_rl_repo")

import numpy as np

import concourse.bass as bass
import concourse.tile as tile
from concourse import bacc, mybir
from concourse.bass_utils import run_bass_kernel_spmd

F32 = mybir.dt.float32
BF16 = mybir.dt.bfloat16
EXP = mybir.ActivationFunctionType.Exp

B, T, D = 2, 2048, 2048
HQ, HK = 32, 8
DH = D // HQ              # 64 head dim
N_CORES = 8
GROUPS = 4                # head groups per batch
QCOLS = D // GROUPS       # 512 q cols per core
KCOLS = (D // 4) // GROUPS  # 128 k cols per core (2 kv heads)
WCOLS = QCOLS + 2 * KCOLS   # 768
TB = 512                  # phase-1 token block == attention query block
NTB = T // TB             # 4
KT = D // 128             # 16 contraction tiles

_cache = {}


def _build():
    nc = bacc.Bacc("TRN2", target_bir_lowering=False, debug=False)

    xT_d = nc.declare_dram_parameter("xT", [D, T], BF16, isOutput=False)
    wqkv_d = nc.declare_dram_parameter("wqkv", [128, KT * WCOLS], BF16,
                                       isOutput=False)
    oproj_d = nc.declare_dram_parameter("oproj", [QCOLS, D], BF16, isOutput=False)
    # tri01[key, q] = 1 if key <= q else 0, replicated for both heads
    zpat_d = nc.declare_dram_parameter("zpat", [128, 2, 128], BF16, isOutput=False)
    ident_d = nc.declare_dram_parameter("ident", [128, 64], F32, isOutput=False)
    ones_d = nc.declare_dram_parameter("ones", [128, 16 * 65], BF16, isOutput=False)
    out_d = nc.declare_dram_parameter("out", [T, D], F32, isOutput=True)
    rcscr_d = nc.dram_tensor("rcscratch", [16, 1024], F32)

    with tile.TileContext(nc) as tc:
        with (
            tc.tile_pool(name="pers", bufs=1) as pers,
            tc.tile_pool(name="xt", bufs=32) as xtp,
            tc.tile_pool(name="work", bufs=2) as work,
            tc.tile_pool(name="psum", bufs=1, space="PSUM") as psum,
        ):
            # ---- persistent SBUF ----
            # xt tiles span 1024 tokens (2KB/partition lines = full DMA
            # efficiency); phase-1 consumes 512-token halves
            wqkv_sb = pers.tile([128, KT, WCOLS], BF16, tag="wqkv")
            zpat_sb = pers.tile([128, 2, 128], BF16, tag="zpat")
            ident_sb = pers.tile([128, 64], F32, tag="ident")
            qt = pers.tile([128, 4, T], BF16, tag="qt")
            # kpz[kv][:, h2, :]: k dims at partitions 64*h2..64*h2+64,
            # zeros elsewhere -> full-128-row S stationaries
            kpz = [pers.tile([128, 2, T], BF16, tag=f"kpz{k}", name=f"kpz{k}")
                   for k in range(2)]
            vT = pers.tile([128, T], F32, tag="vT")
            vs = [pers.tile([128, 16 * 65], BF16, tag=f"vs{k}", name=f"vs{k}")
                  for k in range(2)]
            ctx = pers.tile([128, 4, T], BF16, tag="ctx")
            oproj_sb = pers.tile([128, 4, D], BF16, tag="oproj")

            # tiny constants first so the PE warmup can start immediately
            nc.sync.dma_start(vs[0], ones_d[:])
            nc.sync.dma_start(vs[1], ones_d[:])
            nc.sync.dma_start(zpat_sb, zpat_d[:])
            nc.sync.dma_start(ident_sb, ident_d[:])
            nc.gpsimd.memset(kpz[0][:], 0.0)
            nc.gpsimd.memset(kpz[1][:], 0.0)

            # PE warmup: boost the clock while the startup DMAs stream
            for _ in range(16):
                wps = psum.tile([128, TB], F32, tag="pp", bufs=2, name="wps")
                nc.tensor.matmul(wps[:, 0:512], vs[0][:, 0:128],
                                 vs[0][:, 0:512], start=True, stop=True)

            # weights + first token group, interleaved per contraction tile
            # so the first phase-1 chain starts after ~2 tiles land
            xtiles = {}
            for k in range(KT):
                nc.sync.dma_start(
                    wqkv_sb[:, k, :],
                    wqkv_d[:, k * WCOLS : (k + 1) * WCOLS],
                )
                xt = xtp.tile([128, 1024], BF16, tag="xt", bufs=32, name="xt")
                nc.sync.dma_start(xt, xT_d[128 * k : 128 * k + 128, 0:1024])
                xtiles[(0, k)] = xt
            for k in range(KT):
                xt = xtp.tile([128, 1024], BF16, tag="xt", bufs=32, name="xt")
                nc.sync.dma_start(xt, xT_d[128 * k : 128 * k + 128, 1024:2048])
                xtiles[(1, k)] = xt

            def phase1_block(tb):
                """qkv projection for tokens [TB*tb, TB*tb+TB)."""
                ts = slice(tb * TB, tb * TB + TB)
                g, half = divmod(tb, 2)
                xts = [xtiles[(g, k)][:, 512 * half : 512 * half + 512]
                       for k in range(KT)]
                for m in range(6):
                    ps = psum.tile([128, TB], F32, tag="pp", bufs=2)
                    for k in range(KT):
                        nc.tensor.matmul(
                            ps,
                            wqkv_sb[:, k, 128 * m : 128 * m + 128],
                            xts[k],
                            start=(k == 0),
                            stop=(k == KT - 1),
                        )
                    if m < 4:
                        nc.scalar.copy(qt[:, m, ts], ps)
                    elif m == 4:
                        nc.scalar.copy(kpz[0][0:64, 0, ts], ps[0:64, :])
                        nc.scalar.copy(kpz[1][64:128, 1, ts], ps[64:128, :])
                    else:
                        nc.scalar.copy(vT[:, ts], ps)
                # k duplicate at the other partition half (slot h2^1)
                nc.sync.dma_start(kpz[0][64:128, 1, ts], kpz[0][0:64, 0, ts])
                nc.sync.dma_start(kpz[1][0:64, 0, ts], kpz[1][64:128, 1, ts])
                # v transposes for these 4 key chunks
                for kv in range(2):
                    rows = slice(64 * kv, 64 * kv + 64)
                    for c in range(4 * tb, 4 * tb + 4):
                        tp = psum.tile([128, TB], F32, tag="pp", bufs=2,
                                       name="tp")
                        nc.tensor.transpose(
                            tp[:, 0:64],
                            vT[rows, 128 * c : 128 * c + 128],
                            ident_sb[rows, :],
                            tile_position=(64 * kv, 0),
                        )
                        nc.vector.tensor_copy(
                            vs[kv][:, 65 * c : 65 * c + 64], tp[:, 0:64]
                        )

            def attention_block(a):
                """S/exp/AV + normalize for query block a, all 4 head pairs."""
                nj = 4 * (a + 1)
                isl = slice(512 * a, 512 * a + 512)
                for m in range(4):
                    kv = m // 2
                    ctxAB = psum.tile([65, 1024], F32, tag="cab", bufs=1)
                    pend = []
                    for jc in range(nj):
                        o = jc - 4 * a
                        lo = 128 * o if o >= 0 else 0
                        jsl = slice(128 * jc, 128 * jc + 128)
                        S = psum.tile([128, 2, 512], F32, tag="s2", bufs=2)
                        for h2 in range(2):
                            nc.tensor.matmul(
                                S[:, h2 : h2 + 1, lo:512],
                                kpz[kv][:, h2, jsl],
                                qt[:, m, 512 * a + lo : 512 * a + 512],
                                start=True,
                                stop=True,
                            )
                        E = work.tile([128, 2, 512], BF16, tag="E", bufs=6)
                        nc.scalar.activation(
                            E[:, :, lo:512], S[:, :, lo:512], EXP, scale=0.125
                        )
                        if o >= 0:
                            tri = 128 * o
                            nc.gpsimd.tensor_mul(
                                E[:, :, tri : tri + 128],
                                E[:, :, tri : tri + 128],
                                zpat_sb[:],
                            )
                        pend.append((E, jc, lo))
                        if len(pend) > 2:
                            pE, pjc, plo = pend.pop(0)
                            for h2 in range(2):
                                nc.tensor.matmul(
                                    ctxAB[:, 512 * h2 + plo : 512 * h2 + 512],
                                    vs[kv][:, 65 * pjc : 65 * pjc + 65],
                                    pE[:, h2 : h2 + 1, plo:512],
                                    start=(pjc == 0),
                                    stop=False,
                                )
                    while pend:
                        pE, pjc, plo = pend.pop(0)
                        for h2 in range(2):
                            nc.tensor.matmul(
                                ctxAB[:, 512 * h2 + plo : 512 * h2 + 512],
                                vs[kv][:, 65 * pjc : 65 * pjc + 65],
                                pE[:, h2 : h2 + 1, plo:512],
                                start=(pjc == 0),
                                stop=(pjc == nj - 1),
                            )

                    cu = work.tile([65, 1024], F32, tag="cu", bufs=2,
                                   name="cu")
                    nc.vector.tensor_copy(cu, ctxAB)
                    # normalize: reciprocal of the denominator row, broadcast
                    # to 64 partitions on the (idle) gpsimd engine
                    den128 = work.tile([128, 8], F32, tag="d128", bufs=2,
                                       name="den128")
                    nc.sync.dma_start(den128, cu[64:65, :])
                    rcp = work.tile([128, 8], F32, tag="rcp", bufs=2,
                                    name="rcp")
                    nc.vector.reciprocal(rcp, den128)
                    ma = m * 4 + a
                    nc.sync.dma_start(rcscr_d[ma : ma + 1, :], rcp)
                    bcs = work.tile([64, 1024], F32, tag="bcs", bufs=2,
                                    name="bcs")
                    nc.sync.dma_start(
                        bcs, rcscr_d[ma : ma + 1, :].partition_broadcast(64)
                    )
                    nc.vector.tensor_mul(
                        ctx[0:64, m, isl], cu[0:64, 0:512], bcs[:, 0:512]
                    )
                    tmpB = work.tile([64, 512], BF16, tag="tb", bufs=2)
                    nc.vector.tensor_mul(
                        tmpB, cu[0:64, 512:1024], bcs[:, 512:1024]
                    )
                    nc.sync.dma_start(ctx[64:128, m, isl], tmpB)

            def phase3_block(a, last=False):
                """o-proj for tokens [512a, 512a+512), direct from ctx."""
                for i, t in enumerate(range(4 * a, 4 * a + 4)):
                    tsl = slice(128 * t, 128 * t + 128)
                    for r in range(4):
                        if last and (i * 4 + r) % 2 == 1:
                            po2 = psum.tile([128, 2, 512], F32, tag="s2",
                                            bufs=2, name="po2")
                            po = po2[:, 0, :]
                        else:
                            po = psum.tile([128, TB], F32, tag="pp", bufs=2,
                                           name="po")
                        for m in range(4):
                            nc.tensor.matmul(
                                po,
                                ctx[:, m, tsl],
                                oproj_sb[:, m, 512 * r : 512 * r + 512],
                                start=(m == 0),
                                stop=(m == 3),
                            )
                        og = work.tile([128, TB], F32, tag="og", bufs=3,
                                       name="og")
                        if last:
                            nc.scalar.copy(og, po)
                        else:
                            nc.vector.tensor_copy(og, po)
                        nc.sync.dma_start(
                            out_d[tsl, 512 * r : 512 * r + 512], og
                        )

            for a in range(NTB):
                phase1_block(a)
                attention_block(a)
                if a == 0:
                    nc.sync.dma_start(
                        oproj_sb,
                        oproj_d[:].rearrange("(m p) c -> p m c", p=128),
                    )
                if a >= 1:
                    phase3_block(a - 1)
            phase3_block(NTB - 1, last=True)

    nc.compile()
    return nc


def _host_inputs(x, q_proj, k_proj, v_proj, o_proj):
    """Per-core input dicts."""
    import ml_dtypes

    bf = ml_dtypes.bfloat16
    jj = np.arange(128)[:, None]
    ii = np.arange(128)[None, :]
    tri = (jj <= ii).astype(np.float32)  # tri01[key, q]
    zpat = np.stack([tri, tri], axis=1).astype(bf)  # [128, 2, 128]
    ident = np.zeros((128, 64), dtype=np.float32)
    ident[np.arange(64), np.arange(64)] = 1.0
    ident[np.arange(64) + 64, np.arange(64)] = 1.0

    xT = [np.ascontiguousarray(x[b].T).astype(bf) for b in range(B)]
    in_maps = []
    for c in range(N_CORES):
        b, g = divmod(c, GROUPS)
        wqkv = np.concatenate(
            [
                q_proj[QCOLS * g : QCOLS * g + QCOLS].T,
                k_proj[KCOLS * g : KCOLS * g + KCOLS].T,
                v_proj[KCOLS * g : KCOLS * g + KCOLS].T,
            ],
            axis=1,
        )
        wqkv_arr = (
            np.ascontiguousarray(wqkv)
            .reshape(KT, 128, WCOLS)
            .transpose(1, 0, 2)
            .reshape(128, KT * WCOLS)
        )
        in_maps.append(
            {
                "xT": xT[b],
                "wqkv": np.ascontiguousarray(wqkv_arr).astype(bf),
                "oproj": np.ascontiguousarray(
                    o_proj[:, QCOLS * g : QCOLS * g + QCOLS].T
                ).astype(bf),
                "zpat": zpat,
                "ident": ident,
                "ones": np.ones((128, 16 * 65), dtype=bf),
            }
        )
    return in_maps


def run(x, q_proj, k_proj, v_proj, o_proj, trace=False):
    """Run on hardware; returns (output [B,T,D] f32, BassKernelResults)."""
    if "nc" not in _cache:
        _cache["nc"] = _build()
    nc = _cache["nc"]
    in_maps = _host_inputs(x, q_proj, k_proj, v_proj, o_proj)
    res = run_bass_kernel_spmd(
        nc, in_maps, core_ids=list(range(N_CORES)), trace=trace
    )
    parts = [res.results[c]["out"] for c in range(N_CORES)]
    out = np.empty((B, T, D), dtype=np.float32)
    for b in range(B):
        acc = parts[4 * b].astype(np.float64)
        for g in range(1, GROUPS):
            acc += parts[4 * b + g]
        out[b] = acc.astype(np.float32)
    return out, res


def kernel(x, q_proj, k_proj, v_proj, o_proj, hq=None, hk=None, **_unused):
    x = np.asarray(x, dtype=np.float32)
    q_proj = np.asarray(q_proj, dtype=np.float32)
    k_proj = np.asarray(k_proj, dtype=np.float32)
    v_proj = np.asarray(v_proj, dtype=np.float32)
    o_proj = np.asarray(o_proj, dtype=np.float32)
    assert x.shape == (B, T, D), x.shape
    trace = bool(os.environ.get("KERNEL_TRACE"))
    out, _ = run(x, q_proj, k_proj, v_proj, o_proj, trace=trace)
    return out
